# revision 36
# baseline (speedup 1.0000x reference)
"""MiniCPM attention block on 8 Trainium2 NeuronCores.

Sharding: core c handles batch b = c // 4 and the 8 heads
[ (c%4)*8, (c%4)*8 + 8 ) of that batch (tensor-parallel over heads +
data-parallel over batch).  Each core computes a partial output
x @ block-of-Wo.T of shape [S, HID]; the host sums the 4 partials per
batch.  No collectives.

Causal/bf16 path (build_program_il) -- one flat, interleaved PE stream:
the per-token-chunk QKV projection work and the out-projection are
queued as filler thunks that drain one-per-iteration inside the
attention column loop, so PE never idles on ACT's exp round-trip and
stays HAM-warm.  Per core (S=2048, 8 heads of d=64):
  - 1a thunk: qT/kT m-block = (x @ W.T).T on PE; RoPE as 2 DVE muls off
    PSUM (sin host-preshifted), rotate-half via 4 SBUF->SBUF DMAs, GPS
    add (queued 2 thunks later so it never blocks the strict-FIFO GPS
    queue that also runs the causal affine_selects).
  - 1b thunk: v token-tile = x @ Wv.T, stored [tk, 8*65] with a ones
    column per head (gives the softmax denominator for free in AV).
  - attend(pr, j): S.T tiles [tk 128, tq 512] x 2 heads row-packed
    (concurrent via tile_position), exp on ACT ([128,1024] pair-fused,
    scale=1/8), gpsimd affine_select zeroes the causal triangle on
    boundary tiles (which compute only the valid tq suffix), AV
    accumulates (out.T | denom) into a packed [128,1024] psum bank pair.
    Normalize: raw-evict to SBUF (frees the av banks fast -> psav
    bufs=1), reciprocal on a DMA-transposed [128,8] layout (DVE recip is
    8 cyc/elem -- [1,N] would run on one lane), DRAM stride-0 broadcast,
    2 DVE muls.  The very last normalize instead uses exp(-ln Z) on ACT
    + a 1-contraction PE matmul broadcast (no DMA latency in the tail).
  - outproj half: [128,512] column block accumulated over 4 head-pairs,
    evicted ACT/DVE, DMA'd to the fp32 output.  All outproj is deferred
    to column 3 (the exp-heaviest, otherwise ACT-bound column) as its
    PE filler, paced 3-of-4 iterations.
PSUM: S tag 2x[128,1024] + projection/outproj tag 2x[128,512] + av
[128,1024] = 8 banks.  DMA: bulk loads on the scalar HWDGE queue,
rope-swap/normalize/out descriptors on sync, so loads never queue
behind small descriptors.

Schedule refinements over the first-pass interleave (-11% wall):
  - prologue loads issued in global need-order with every tensor's
    k-group descriptors alternated across BOTH HWDGE rings (x1 before
    wo; rings drain FIFO by bytes, so ordering IS arrival time);
  - prologue rope swap as a PE permutation matmul (pm.T @ u, 213ns)
    with the add on DVE reading PSUM -- swap DMAs issued behind the
    bulk loads executed ~25us late and starved qT/kT assembly;
  - the last head-pair's av eviction per column runs on ACT, not the
    congested DVE FIFO (psav bufs=1 gates the next column's AV on it);
  - output stored as bf16 partials (host sums in float64), halving
    16.8MB of store traffic; ostage bufs=4, stores on both rings;
  - final 1/Z: dummy Ln reading the last p tile pins the Ln table
    prefetch after the final exp (an input-free dummy gets HOISTED by
    the scheduler and thrashes tables), Ln reads the denominator row
    straight from av PSUM, per-half Ln/Exp -> bf16 rec -> single-pass
    bf16 broadcast matmuls;
  - final 16 outproj halves software-pipelined: pr0-2 matmuls open 3
    chains ahead of each pr3+evict close, hiding the 1/Z latency.
Known dead ends (measured): fp8 anywhere fails the 2e-2 gate (exp
amplification / peaked-attention value copying); AV's 65-row matmul is
the softmax denominator and cannot be col-packed (65+65 > 128).

Self-contained: shapes hardcoded from the problem spec.
"""
import numpy as np
import ml_dtypes

S = 2048
HID = 2048
NH = 32
DH = 64
N_CORES = 8
HEADS_PER_CORE = NH // (N_CORES // 2)   # 8
BLK = HEADS_PER_CORE * DH               # 512
ROPE_BASE = 10000.0

_PROGRAMS = {}


def _rope_cache():
    inv_freq = 1.0 / (ROPE_BASE ** (np.arange(0, DH, 2, dtype=np.float32) / DH))
    t = np.arange(S, dtype=np.float32)
    freqs = np.outer(t, inv_freq)                     # [S, 32]
    emb = np.concatenate([freqs, freqs], axis=-1)     # [S, 64]
    return np.cos(emb), np.sin(emb)


def build_program(variant="causal", dtype="bf16"):
    """Build the Bacc program (one NEFF, run SPMD on 8 cores)."""
    import concourse.bass as bass
    import concourse.mybir as mybir
    import concourse.tile as tile
    from concourse import bacc

    fp32 = mybir.dt.float32
    if dtype == "bf16":
        DT = mybir.dt.bfloat16      # dram/lhs/rhs matmul dtype
        SDT = mybir.dt.bfloat16     # on-chip storage for q/k/v/p/attn
        CH = 512                    # token chunk for phase 1
        MDT = DT                    # matmul ap dtype (no bitcast needed)
    elif dtype == "fp32r":
        DT = mybir.dt.float32
        SDT = mybir.dt.float32
        CH = 256
        MDT = mybir.dt.float32r
    else:
        raise ValueError(dtype)

    def mm(ap):
        return ap.bitcast(MDT) if MDT is not ap.dtype else ap

    causal = variant == "causal"
    NCH = S // CH            # phase-1 token chunks
    NT = S // 128            # 16 token tiles
    NPR = 4                  # head pairs

    nc = bacc.Bacc("TRN2", target_bir_lowering=False, debug=False,
                   enable_asserts=False, num_devices=N_CORES)

    xT = nc.dram_tensor("xT", [HID, S], DT, kind="ExternalInput").ap()
    wqT = nc.dram_tensor("wqT", [HID, BLK], DT, kind="ExternalInput").ap()
    wkT = nc.dram_tensor("wkT", [HID, BLK], DT, kind="ExternalInput").ap()
    wvT = nc.dram_tensor("wvT", [HID, BLK], DT, kind="ExternalInput").ap()
    woT = nc.dram_tensor("woT", [BLK, HID], DT, kind="ExternalInput").ap()
    cos2 = nc.dram_tensor("cos2", [128, S], fp32, kind="ExternalInput").ap()
    # sin, pre-multiplied by the rotate-half sign AND partition-swapped so
    # that u = ps * sin_sh followed by a 32-row partition swap of u equals
    # rot(ps) * sin  (the swap moves to a DMA, freeing ACT for exp).
    sin2s = nc.dram_tensor("sin2s", [128, S], fp32, kind="ExternalInput").ap()
    if not causal:
        maskT = nc.dram_tensor("maskT", [S, S], mybir.dt.bfloat16,
                               kind="ExternalInput").ap()
    out = nc.dram_tensor("out", [S, HID], fp32, kind="ExternalOutput").ap()

    tc_ctx = tile.TileContext(nc)

    def phase1_bf16(tc, qT_sb, kT_sb, v_sb):
        with tc.tile_pool(name="consts", bufs=1) as cpool, \
             tc.tile_pool(name="wall", bufs=1) as wpool, \
             tc.tile_pool(name="xa", bufs=2) as xa, \
             tc.tile_pool(name="ropetmp", bufs=2) as rt:
            cos_sb = cpool.tile([128, S], fp32)
            sin_sb = cpool.tile([128, S], fp32)
            nc.sync.dma_start(out=cos_sb, in_=cos2)
            nc.sync.dma_start(out=sin_sb, in_=sin2s)
            wq_sb = wpool.tile([128, 16, BLK], DT)
            wk_sb = wpool.tile([128, 16, BLK], DT)
            wv_sb = wpool.tile([128, 16, BLK], DT)
            for w_sb, w_d in ((wq_sb, wqT), (wk_sb, wkT), (wv_sb, wvT)):
                wd = w_d.rearrange("(k p) m -> p k m", p=128)
                for kg in range(4):
                    nc.sync.dma_start(out=w_sb[:, 4 * kg:4 * kg + 4, :],
                                      in_=wd[:, 4 * kg:4 * kg + 4, :])

            # --- 1a: Q/K + RoPE ---
            with tc.tile_pool(name="psqk", bufs=1, space="PSUM") as psqk:
                for n in range(NCH):
                    sl = slice(n * CH, (n + 1) * CH)
                    x_ch = xa.tile([128, 16, CH], DT, name=f"x1a_{n}", tag="x")
                    xd = xT[:, sl].rearrange("(k p) t -> p k t", p=128)
                    for kg in range(4):
                        nc.sync.dma_start(out=x_ch[:, 4 * kg:4 * kg + 4, :],
                                          in_=xd[:, 4 * kg:4 * kg + 4, :])
                    for w_sb, dst, wn in ((wq_sb, qT_sb, "q"), (wk_sb, kT_sb, "k")):
                        for m in range(NPR):
                            ps = psqk.tile([128, CH], fp32,
                                           name=f"ps{wn}{m}_{n}", tag=f"ps{wn}{m}")
                            for k in range(16):
                                nc.tensor.matmul(
                                    ps,
                                    lhsT=mm(w_sb[:, k, m * 128:(m + 1) * 128]),
                                    rhs=mm(x_ch[:, k, :]),
                                    start=(k == 0), stop=(k == 15))
                            # rope: 2 full-width DVE muls off PSUM, then the
                            # rotate-half partition swap via 4 SBUF->SBUF
                            # DMAs (sin is host-preshifted so swap-after-mul
                            # equals mul-after-swap), then GPS add.  ACT does
                            # no rope work at all -- it is exp-bound later.
                            u = rt.tile([128, CH], fp32, name=f"u{wn}{m}_{n}", tag="u")
                            t1 = rt.tile([128, CH], fp32, name=f"t1{wn}{m}_{n}", tag="t1")
                            ru = rt.tile([128, CH], fp32, name=f"ru{wn}{m}_{n}", tag="ru")
                            nc.vector.tensor_mul(t1, ps, cos_sb[:, sl])
                            nc.vector.tensor_mul(u, ps, sin_sb[:, sl])
                            for (d, s_) in ((0, 32), (32, 0), (64, 96), (96, 64)):
                                nc.sync.dma_start(out=ru[d:d + 32, :],
                                                  in_=u[s_:s_ + 32, :])
                            nc.gpsimd.tensor_add(dst[:, m, sl], t1, ru)

            # --- 1b: V (x pool shared; weights already loaded) ---
            with tc.tile_pool(name="psv", bufs=2, space="PSUM") as psv:
                for n in range(NCH):
                    sl = slice(n * CH, (n + 1) * CH)
                    x_ch = xa.tile([128, 16, CH], DT, name=f"x1b_{n}", tag="x")
                    nc.sync.dma_start(out=x_ch,
                                      in_=xT[:, sl].rearrange("(k p) t -> p k t", p=128))
                    for s_ in range(CH // 128):
                        t16 = (n * CH) // 128 + s_
                        ps = psv.tile([128, BLK], fp32, name=f"psv{t16}", tag="psv")
                        for k in range(16):
                            nc.tensor.matmul(
                                ps,
                                lhsT=mm(x_ch[:, k, s_ * 128:(s_ + 1) * 128]),
                                rhs=mm(wv_sb[:, k, :]),
                                start=(k == 0), stop=(k == 15))
                        dst = v_sb[:, t16, :].rearrange("p (h c) -> p h c", c=65)[:, :, 0:64]
                        nc.scalar.copy(dst, ps.rearrange("p (h c) -> p h c", c=64))

    def phase1_fp32r(tc, qT_sb, kT_sb, v_pool_open):
        # 1a with only Wq/Wk resident; v_sb allocated after (SBUF pressure)
        with tc.tile_pool(name="consts", bufs=1) as cpool, \
             tc.tile_pool(name="wqk", bufs=1) as wpool, \
             tc.tile_pool(name="xa", bufs=2) as xa, \
             tc.tile_pool(name="ropetmp", bufs=2) as rt, \
             tc.tile_pool(name="psqk", bufs=1, space="PSUM") as psqk:
            cos_sb = cpool.tile([128, S], fp32)
            sin_sb = cpool.tile([128, S], fp32)
            nc.sync.dma_start(out=cos_sb, in_=cos2)
            nc.sync.dma_start(out=sin_sb, in_=sin2s)
            wq_sb = wpool.tile([128, 16, BLK], DT)
            wk_sb = wpool.tile([128, 16, BLK], DT)
            nc.sync.dma_start(out=wq_sb, in_=wqT.rearrange("(k p) m -> p k m", p=128))
            nc.sync.dma_start(out=wk_sb, in_=wkT.rearrange("(k p) m -> p k m", p=128))
            for n in range(NCH):
                sl = slice(n * CH, (n + 1) * CH)
                x_ch = xa.tile([128, 16, CH], DT, name=f"x1a_{n}", tag="x")
                nc.sync.dma_start(out=x_ch,
                                  in_=xT[:, sl].rearrange("(k p) t -> p k t", p=128))
                for w_sb, dst, wn in ((wq_sb, qT_sb, "q"), (wk_sb, kT_sb, "k")):
                    for m in range(NPR):
                        ps = psqk.tile([128, CH], fp32,
                                       name=f"ps{wn}{m}_{n}", tag=f"ps{wn}{m}")
                        for k in range(16):
                            nc.tensor.matmul(
                                ps,
                                lhsT=mm(w_sb[:, k, m * 128:(m + 1) * 128]),
                                rhs=mm(x_ch[:, k, :]),
                                start=(k == 0), stop=(k == 15))
                        u = rt.tile([128, CH], fp32, name=f"u{wn}{m}_{n}", tag="u")
                        t1 = rt.tile([128, CH], fp32, name=f"t1{wn}{m}_{n}", tag="t1")
                        ru = rt.tile([128, CH], fp32, name=f"ru{wn}{m}_{n}", tag="ru")
                        nc.vector.tensor_mul(t1, ps, cos_sb[:, sl])
                        nc.vector.tensor_mul(u, ps, sin_sb[:, sl])
                        for (d, s_) in ((0, 32), (32, 0), (64, 96), (96, 64)):
                            nc.sync.dma_start(out=ru[d:d + 32, :],
                                              in_=u[s_:s_ + 32, :])
                        nc.gpsimd.tensor_add(dst[:, m, sl], t1, ru)

        v_sb = v_pool_open()
        with tc.tile_pool(name="wv", bufs=1) as wvp, \
             tc.tile_pool(name="xb", bufs=2) as xb, \
             tc.tile_pool(name="psv", bufs=2, space="PSUM") as psv:
            wv_sb = wvp.tile([128, 16, BLK], DT)
            nc.sync.dma_start(out=wv_sb, in_=wvT.rearrange("(k p) m -> p k m", p=128))
            for n in range(NCH):
                sl = slice(n * CH, (n + 1) * CH)
                x_ch = xb.tile([128, 16, CH], DT, name=f"x1b_{n}", tag="x")
                nc.sync.dma_start(out=x_ch,
                                  in_=xT[:, sl].rearrange("(k p) t -> p k t", p=128))
                for s_ in range(CH // 128):
                    t16 = (n * CH) // 128 + s_
                    ps = psv.tile([128, BLK], fp32, name=f"psv{t16}", tag="psv")
                    for k in range(16):
                        nc.tensor.matmul(
                            ps,
                            lhsT=mm(x_ch[:, k, s_ * 128:(s_ + 1) * 128]),
                            rhs=mm(wv_sb[:, k, :]),
                            start=(k == 0), stop=(k == 15))
                    dst = v_sb[:, t16, :].rearrange("p (h c) -> p h c", c=65)[:, :, 0:64]
                    nc.scalar.copy(dst, ps.rearrange("p (h c) -> p h c", c=64))
        return v_sb

    with tc_ctx as tc:
        with tc.tile_pool(name="qk_sb", bufs=1) as qk_pool:
            qT_sb = qk_pool.tile([128, NPR, S], SDT)
            kT_sb = qk_pool.tile([128, NPR, S], SDT)

            import contextlib
            vstack = contextlib.ExitStack()
            with vstack:
                def v_pool_open():
                    v_pool = vstack.enter_context(tc.tile_pool(name="v_sb_pool", bufs=1))
                    v_sb = v_pool.tile([128, NT, HEADS_PER_CORE * 65], SDT)
                    ones_ap = v_sb.rearrange("p t (h c) -> p t h c", c=65)[:, :, :, 64:65]
                    nc.vector.memset(ones_ap, 1.0)
                    return v_sb

                if dtype == "bf16":
                    v_sb = v_pool_open()
                    phase1_bf16(tc, qT_sb, kT_sb, v_sb)
                else:
                    v_sb = phase1_fp32r(tc, qT_sb, kT_sb, v_pool_open)

                # -------- phase 2+3: attention + fused out-projection ----
                with tc.tile_pool(name="attn_pool", bufs=1) as apool, \
                     tc.tile_pool(name="wo", bufs=1) as wop:
                    attn_sb = apool.tile([128, NPR, S], SDT)
                    wo_sb = wop.tile([128, NPR, HID], DT)
                    nc.sync.dma_start(out=wo_sb,
                                      in_=woT.rearrange("(r p) o -> p r o", p=128))
                    with tc.tile_pool(name="ppool", bufs=3) as ppool, \
                         tc.tile_pool(name="npool", bufs=2) as npool, \
                         tc.tile_pool(name="mpool", bufs=2) as mpool, \
                         tc.tile_pool(name="ostage", bufs=2) as ostage, \
                         tc.tile_pool(name="dpool", bufs=2, space="DRAM") as dpool, \
                         tc.tile_pool(name="pss", bufs=2, space="PSUM") as pss, \
                         tc.tile_pool(name="psav", bufs=2, space="PSUM") as psav:

                        from collections import deque
                        pending_out = deque()

                        def emit_outproj_group(t16, nop):
                            # 2x512 output columns into one 2-bank pss slot
                            o_ps = pss.tile([128, 1024], fp32,
                                            name=f"o{t16}_{nop}", tag="s")
                            for half in range(2):
                                no = nop * 2 + half
                                for pr_ in range(NPR):
                                    nc.tensor.matmul(
                                        o_ps[:, half * 512:(half + 1) * 512],
                                        lhsT=mm(attn_sb[:, pr_,
                                                        t16 * 128:(t16 + 1) * 128]),
                                        rhs=mm(wo_sb[:, pr_,
                                                     no * 512:(no + 1) * 512]),
                                        start=(pr_ == 0), stop=(pr_ == NPR - 1))
                            o_sb = ostage.tile([128, 1024], fp32,
                                               name=f"os{t16}_{nop}", tag="os")
                            if nop % 2 == 0:
                                nc.scalar.copy(o_sb, o_ps)
                            else:
                                nc.vector.tensor_copy(o_sb, o_ps)
                            nc.sync.dma_start(
                                out=out[t16 * 128:(t16 + 1) * 128,
                                        nop * 1024:(nop + 1) * 1024],
                                in_=o_sb)

                        def drain_one():
                            if pending_out:
                                t16, no = pending_out.popleft()
                                emit_outproj_group(t16, no)

                        def attend(pr, j, mask_col):
                            n_i = 4 * j + 4 if causal else NT
                            # one packed AV tile (2 banks): head half=0 in
                            # cols 0:512, half=1 in cols 512:1024; rows 0-64
                            # accumulate (out.T | denom).
                            av = psav.tile([128, 1024], fp32,
                                           name=f"av_{pr}_{j}", tag="av")
                            s_t, p_t, nw_t = {}, {}, {}

                            def emit_S(i):
                                # causal boundary tiles only need the tq
                                # suffix [512j+off, 512(j+1))
                                off = max(0, 128 * (i - 4 * j)) if causal else 0
                                nw = 512 - off
                                nw_t[i] = (off, nw)
                                s_ps = pss.tile([128, 1024], fp32,
                                                name=f"s_{pr}_{j}_{i}", tag="s")
                                for half in range(2):
                                    r0 = 64 * half
                                    nc.tensor.matmul(
                                        s_ps[:, half * 512:half * 512 + nw],
                                        lhsT=mm(kT_sb[r0:r0 + 64, pr,
                                                      i * 128:(i + 1) * 128]),
                                        rhs=mm(qT_sb[r0:r0 + 64, pr,
                                                     j * 512 + off:(j + 1) * 512]),
                                        start=True, stop=True,
                                        tile_position=(r0, 0))
                                s_t[i] = s_ps

                            def emit_exp(i):
                                off, nw = nw_t[i]
                                s_ps = s_t[i]
                                s_v = s_ps.rearrange("q (h t) -> q h t", h=2)[:, :, 0:nw]
                                p = ppool.tile([128, 1024], SDT,
                                               name=f"p_{pr}_{j}_{i}", tag="p")
                                p_v = p.rearrange("q (h t) -> q h t", h=2)[:, :, 0:nw]
                                if causal:
                                    nc.scalar.activation(p_v, s_v,
                                                         mybir.ActivationFunctionType.Exp,
                                                         scale=0.125)
                                    if i >= 4 * j:
                                        # keep iff tq - tk >= 0 (base is 0 on
                                        # boundary tiles thanks to the suffix)
                                        nc.gpsimd.affine_select(
                                            out=p_v, in_=p_v,
                                            compare_op=mybir.AluOpType.is_ge,
                                            fill=0.0,
                                            base=512 * j + off - 128 * i,
                                            pattern=[[0, 2], [1, nw]],
                                            channel_multiplier=-1)
                                else:
                                    tmp = ppool.tile([128, 1024], fp32,
                                                     name=f"pt_{pr}_{j}_{i}", tag="pt")
                                    for half in range(2):
                                        nc.vector.scalar_tensor_tensor(
                                            out=tmp[:, half * 512:(half + 1) * 512],
                                            in0=s_ps[:, half * 512:(half + 1) * 512],
                                            scalar=0.125,
                                            in1=mask_col[:, i, :],
                                            op0=mybir.AluOpType.mult,
                                            op1=mybir.AluOpType.add)
                                    nc.scalar.activation(p, tmp,
                                                         mybir.ActivationFunctionType.Exp)
                                p_t[i] = p

                            def emit_AV(i, first, last):
                                off, nw = nw_t[i]
                                p = p_t[i]
                                for half in range(2):
                                    h = 2 * pr + half
                                    nc.tensor.matmul(
                                        av[0:65, half * 512 + off:half * 512 + 512],
                                        lhsT=mm(v_sb[:, i, 65 * h:65 * h + 65]),
                                        rhs=mm(p[:, half * 512:half * 512 + nw]),
                                        start=first, stop=last)

                            # software pipeline: S runs 2 tiles ahead of AV,
                            # with outproj matmul groups drained between
                            # iterations as PE filler while ACT runs exp.
                            # (ascending i is required: AV i=0 writes the full
                            # 512 with start=True, clearing has_written before
                            # the suffix-trimmed boundary tiles accumulate)
                            order = list(range(n_i))
                            first_i, last_i = order[0], order[-1]
                            emit_S(order[0])
                            if n_i > 1:
                                emit_S(order[1])
                            emit_exp(order[0])
                            for ii, i in enumerate(order):
                                if ii + 2 < n_i:
                                    emit_S(order[ii + 2])
                                if ii + 1 < n_i:
                                    emit_exp(order[ii + 1])
                                emit_AV(i, i == first_i, i == last_i)
                                drain_one()

                            # normalize, off the critical path: raw-evict
                            # (out.T | denom) to SBUF in one copy (frees the
                            # av banks ~1.2us after the last AV), then the
                            # reciprocal on a DMA-transposed [128, 8] layout
                            # (DVE recip is 8 cyc/elem, so [1, N] shapes on
                            # one lane are ~8.5us -- transposed it is ~0.2us),
                            # broadcast back via DRAM with a stride-0 AP.
                            raw = npool.tile([65, 1024], fp32,
                                             name=f"raw_{pr}_{j}", tag="raw")
                            nc.vector.tensor_copy(raw, av[0:65, :])
                            den_d = dpool.tile([1, 1024], fp32,
                                               name=f"dd_{pr}_{j}", tag="dd")
                            nc.sync.dma_start(out=den_d, in_=raw[64:65, :])
                            den_t = npool.tile([128, 8], fp32,
                                               name=f"dt_{pr}_{j}", tag="dt")
                            nc.sync.dma_start(
                                out=den_t,
                                in_=den_d.rearrange("o (p c) -> (o p) c", c=8))
                            rec_t = npool.tile([128, 8], fp32,
                                               name=f"rt_{pr}_{j}", tag="rt")
                            nc.vector.reciprocal(rec_t, den_t)
                            rec_d = dpool.tile([128, 8], fp32,
                                               name=f"rd_{pr}_{j}", tag="rd")
                            nc.sync.dma_start(out=rec_d, in_=rec_t)
                            jsl = slice(j * 512, (j + 1) * 512)
                            for half in range(2):
                                bc = npool.tile([64, 512], fp32,
                                                name=f"bc{half}_{pr}_{j}",
                                                tag=f"bc{half}")
                                bc_src = bass.AP(
                                    tensor=rec_d.tensor,
                                    offset=rec_d.offset + half * 512,
                                    ap=[[0, 64], [1, 512]])
                                nc.sync.dma_start(out=bc, in_=bc_src)
                                nc.vector.tensor_mul(
                                    attn_sb[64 * half:64 * half + 64, pr, jsl],
                                    raw[0:64, half * 512:half * 512 + 512], bc)

                        for j in range(4):
                            if causal:
                                mask_col = None
                            else:
                                mask_col = mpool.tile([128, NT, 512],
                                                      mybir.dt.bfloat16,
                                                      name=f"mc{j}", tag="mc")
                                nc.sync.dma_start(
                                    out=mask_col,
                                    in_=maskT[:, j * 512:(j + 1) * 512]
                                    .rearrange("(i p) t -> p i t", p=128))
                            for pr in range(NPR):
                                attend(pr, j, mask_col)
                            # column j's attention rows are complete; queue
                            # its out-projection as PE filler for column j+1
                            for t16 in range(4 * j, 4 * j + 4):
                                for nop in range(2):
                                    pending_out.append((t16, nop))
                        while pending_out:
                            drain_one()
    nc.compile()
    return nc


def build_program_il(dtype="bf16"):
    """Interleaved causal-only builder.

    One flat PE stream: the Q/K/V projection work for token-chunk n+1 and
    the out-projection for column n-1 are queued as filler thunks that
    drain one-per-iteration inside attention column n's i-loop.  PE never
    idles waiting for ACT's exp round-trip (it runs filler matmuls), stays
    HAM-warm, and exp for column n overlaps projection matmuls.

    PSUM budget (8 banks): S tiles tag "s" [128,1024] x2 bufs (4 banks),
    projection tiles tag "qk" [128,512] x2 (2 banks), packed AV
    accumulator tag "av" [128,1024] x1 (2 banks; freed ~1.2us after the
    last AV by the raw evict, so bufs=1 does not stall).

    DMA queues: bulk loads (x, weights, cos/sin) go on the scalar HWDGE
    queue so they are never stuck behind the many small rope-swap /
    normalize descriptors on the sync queue.
    """
    import concourse.bass as bass
    import concourse.mybir as mybir
    import concourse.tile as tile
    from concourse import bacc
    from collections import deque

    fp32 = mybir.dt.float32
    bf16 = mybir.dt.bfloat16
    DT = bf16
    SDT = bf16
    CH = 512
    NCH = S // CH            # 4 token chunks == 4 tq columns
    NT = S // 128            # 16 token tiles
    NPR = 4                  # head pairs

    nc = bacc.Bacc("TRN2", target_bir_lowering=False, debug=False,
                   enable_asserts=False, num_devices=N_CORES)

    xT = nc.dram_tensor("xT", [HID, S], DT, kind="ExternalInput").ap()
    wqT = nc.dram_tensor("wqT", [HID, BLK], DT, kind="ExternalInput").ap()
    wkT = nc.dram_tensor("wkT", [HID, BLK], DT, kind="ExternalInput").ap()
    wvT = nc.dram_tensor("wvT", [HID, BLK], DT, kind="ExternalInput").ap()
    woT = nc.dram_tensor("woT", [BLK, HID], DT, kind="ExternalInput").ap()
    cosb = nc.dram_tensor("cosb", [128, S], bf16, kind="ExternalInput").ap()
    sinb = nc.dram_tensor("sinb", [128, S], bf16, kind="ExternalInput").ap()
    # rotate-half permutation matrix: pm[k, m] = 1 iff k == swap(m), so
    # (pm.T @ u)[m] = u[swap(m)] -- used by the prologue to run the rope
    # partition swap on the PE (213ns) instead of the load-saturated rings.
    pmT = nc.dram_tensor("pm", [128, 128], bf16, kind="ExternalInput").ap()
    # bf16 partials: the host sums the 4 per-batch partials in float64, so
    # bf16 quantization of each partial adds ~3e-3 abs (vs the 5.2e-2 gate)
    # while halving 16.8MB of store traffic per core.
    out = nc.dram_tensor("out", [S, HID], bf16, kind="ExternalOutput").ap()

    tc_ctx = tile.TileContext(nc)
    with tc_ctx as tc:
        with tc.tile_pool(name="consts", bufs=1) as cpool, \
             tc.tile_pool(name="wall", bufs=1) as wpool, \
             tc.tile_pool(name="qk_sb", bufs=1) as qk_pool, \
             tc.tile_pool(name="v_sb_pool", bufs=1) as vpool, \
             tc.tile_pool(name="attn_pool", bufs=1) as apool, \
             tc.tile_pool(name="xa", bufs=2) as xa, \
             tc.tile_pool(name="ropetmp", bufs=4) as rt, \
             tc.tile_pool(name="ropelag", bufs=3) as rtl, \
             tc.tile_pool(name="ppool", bufs=4) as ppool, \
             tc.tile_pool(name="npool", bufs=2) as npool, \
             tc.tile_pool(name="bcpool", bufs=1) as bcpool, \
             tc.tile_pool(name="ostage", bufs=4) as ostage, \
             tc.tile_pool(name="dpool", bufs=2, space="DRAM") as dpool, \
             tc.tile_pool(name="pss", bufs=2, space="PSUM") as pss, \
             tc.tile_pool(name="psqk", bufs=2, space="PSUM") as psqk, \
             tc.tile_pool(name="psav", bufs=1, space="PSUM") as psav:

            cos_sb = cpool.tile([128, S], bf16)
            sin_sb = cpool.tile([128, S], bf16)
            wq_sb = wpool.tile([128, 16, BLK], DT)
            wk_sb = wpool.tile([128, 16, BLK], DT)
            wv_sb = wpool.tile([128, 16, BLK], DT)
            wo_sb = wpool.tile([128, NPR, HID], DT)
            qT_sb = qk_pool.tile([128, NPR, S], SDT)
            kT_sb = qk_pool.tile([128, NPR, S], SDT)
            v_sb = vpool.tile([128, NT, HEADS_PER_CORE * 65], SDT)
            attn_sb = apool.tile([128, NPR, S], SDT)

            x_tiles = {}

            def load_x(n, split=False):
                # 4 descriptors for finer dependency granularity (matmul
                # chains start on the first k-group).  split=True alternates
                # the two HWDGE queues so the chunk lands at aggregate HBM
                # rate instead of one queue's.
                xch = xa.tile([128, 16, CH], DT, name=f"x_{n}", tag="x")
                xd = xT[:, n * CH:(n + 1) * CH].rearrange("(k p) t -> p k t", p=128)
                for kg in range(4):
                    eng = (nc.scalar, nc.sync)[kg % 2] if split else nc.scalar
                    eng.dma_start(out=xch[:, 4 * kg:4 * kg + 4, :],
                                  in_=xd[:, 4 * kg:4 * kg + 4, :])
                x_tiles[n] = xch

            # Prologue loads in global need-order -- x0+wq (gate the first
            # 1a chain), cos/sin (rope of thunk 1), wk (thunk 5), wv (1b,
            # ~45us), x1 (col-0 fillers, ~42us), wo (col-3 outproj, ~200us)
            # -- with every tensor's k-group descriptors alternated across
            # BOTH HWDGE queues.  The first ~50us are HBM-bound, so arrival
            # time is set by the aggregate need-order schedule; the baseline
            # serialized wk behind cos/sin on sync (landing ~48us) and x1
            # behind wv+wo on scalar (~50us), starving the prologue at
            # ~37us and dropping the PE HAM clock-gate to cold.
            xch0 = xa.tile([128, 16, CH], DT, name="x_0", tag="x")
            xd0 = xT[:, 0:CH].rearrange("(k p) t -> p k t", p=128)
            wdq = wqT.rearrange("(k p) m -> p k m", p=128)
            # kg0 split into 0.25MB halves so the first 1a chain's gating
            # transfer lands ~2us earlier (the whole phase is DMA-paced).
            # (Finer granularity for ALL descriptors was tried and is
            # neutral-to-worse: descriptor-issue engine time eats the gain.)
            for h in range(2):
                nc.scalar.dma_start(out=xch0[:, 2 * h:2 * h + 2, :],
                                    in_=xd0[:, 2 * h:2 * h + 2, :])
                nc.sync.dma_start(out=wq_sb[:, 2 * h:2 * h + 2, :],
                                  in_=wdq[:, 2 * h:2 * h + 2, :])
            for kg in range(1, 4):
                (nc.scalar, nc.sync)[kg % 2].dma_start(
                    out=xch0[:, 4 * kg:4 * kg + 4, :],
                    in_=xd0[:, 4 * kg:4 * kg + 4, :])
                (nc.sync, nc.scalar)[kg % 2].dma_start(
                    out=wq_sb[:, 4 * kg:4 * kg + 4, :],
                    in_=wdq[:, 4 * kg:4 * kg + 4, :])
            x_tiles[0] = xch0
            pm_sb = cpool.tile([128, 128], bf16)
            nc.sync.dma_start(out=pm_sb, in_=pmT)
            nc.sync.dma_start(out=cos_sb, in_=cosb)
            nc.scalar.dma_start(out=sin_sb, in_=sinb)
            wdk = wkT.rearrange("(k p) m -> p k m", p=128)
            for kg in range(4):
                (nc.scalar, nc.sync)[kg % 2].dma_start(
                    out=wk_sb[:, 4 * kg:4 * kg + 4, :],
                    in_=wdk[:, 4 * kg:4 * kg + 4, :])
            wdv = wvT.rearrange("(k p) m -> p k m", p=128)
            for kg in range(4):
                (nc.sync, nc.scalar)[kg % 2].dma_start(
                    out=wv_sb[:, 4 * kg:4 * kg + 4, :],
                    in_=wdv[:, 4 * kg:4 * kg + 4, :])
            load_x(1, split=True)
            wdo = woT.rearrange("(r p) o -> p r o", p=128)
            nc.sync.dma_start(out=wo_sb[:, 0:2, :], in_=wdo[:, 0:2, :])
            nc.scalar.dma_start(out=wo_sb[:, 2:4, :], in_=wdo[:, 2:4, :])
            ones_ap = v_sb.rearrange("p t (h c) -> p t h c", c=65)[:, :, :, 64:65]
            nc.vector.memset(ones_ap, 1.0)
            ones1b_sb = cpool.tile([1, 64], bf16)
            nc.vector.memset(ones1b_sb, 1.0)

            # ---------------- thunks ----------------
            def make_1a(wn_idx, m, n, swap_eng=None):
                # split: [matmul chain + rope muls + swap DMAs] vs the
                # GPS add.  The add is queued ~2 thunks later so it never
                # sits at the head of the strict-FIFO GPS queue waiting on
                # its swap DMAs (it would block affine_selects behind it,
                # which gate the attention AV path).
                cell = {}

                def main():
                    w_sb_ = (wq_sb, wk_sb)[wn_idx]
                    sl = slice(n * CH, (n + 1) * CH)
                    x_ch = x_tiles[n]
                    ps = psqk.tile([128, CH], fp32,
                                   name=f"ps{wn_idx}_{m}_{n}", tag="qk")
                    for k in range(16):
                        nc.tensor.matmul(ps,
                                         lhsT=w_sb_[:, k, m * 128:(m + 1) * 128],
                                         rhs=x_ch[:, k, :],
                                         start=(k == 0), stop=(k == 15))
                    u = rt.tile([128, CH], bf16, name=f"u{wn_idx}{m}_{n}", tag="u")
                    t1 = rtl.tile([128, CH], bf16, name=f"t1{wn_idx}{m}_{n}", tag="t1")
                    ru = rtl.tile([128, CH], bf16, name=f"ru{wn_idx}{m}_{n}", tag="ru")
                    nc.vector.tensor_mul(t1, ps, cos_sb[:, sl])
                    nc.vector.tensor_mul(u, ps, sin_sb[:, sl])
                    eng = swap_eng or nc.sync
                    for (d, s_) in ((0, 32), (32, 0), (64, 96), (96, 64)):
                        eng.dma_start(out=ru[d:d + 32, :], in_=u[s_:s_ + 32, :])
                    cell["t1"], cell["ru"], cell["sl"] = t1, ru, sl

                def add():
                    dstq = (qT_sb, kT_sb)[wn_idx]
                    nc.gpsimd.tensor_add(dstq[:, m, cell["sl"]],
                                         cell["t1"], cell["ru"])
                return main, add

            def make_1a_pe(wn_idx, m, n):
                # Prologue-only 1a: the rotate-half partition swap runs as a
                # 1-matmul PE permutation (ru = pm.T @ u, 213ns) instead of
                # SBUF->SBUF ring DMAs.  During the first ~50us the HWDGE
                # rings are saturated with bulk loads, so swap descriptors
                # issued behind them execute ~25us late and starve qT/kT
                # assembly (the baseline's first GPS add ran at t=45us).  ru
                # lands in the psav "av" PSUM slot (idle until the first
                # attend); the add runs on DVE, which reads PSUM directly.
                cell = {}

                def main():
                    w_sb_ = (wq_sb, wk_sb)[wn_idx]
                    sl = slice(n * CH, (n + 1) * CH)
                    x_ch = x_tiles[n]
                    ps = psqk.tile([128, CH], fp32,
                                   name=f"ps{wn_idx}_{m}_{n}", tag="qk")
                    for k in range(16):
                        nc.tensor.matmul(ps,
                                         lhsT=w_sb_[:, k, m * 128:(m + 1) * 128],
                                         rhs=x_ch[:, k, :],
                                         start=(k == 0), stop=(k == 15))
                    u = rt.tile([128, CH], bf16, name=f"u{wn_idx}{m}_{n}", tag="u")
                    t1 = rtl.tile([128, CH], bf16, name=f"t1{wn_idx}{m}_{n}", tag="t1")
                    nc.vector.tensor_mul(t1, ps, cos_sb[:, sl])
                    nc.vector.tensor_mul(u, ps, sin_sb[:, sl])
                    cell["t1"], cell["u"], cell["sl"] = t1, u, sl

                def swap_mm():
                    ru_ps = psav.tile([128, CH], fp32,
                                      name=f"rups{wn_idx}{m}_{n}", tag="av")
                    nc.tensor.matmul(ru_ps, lhsT=pm_sb, rhs=cell["u"],
                                     start=True, stop=True)
                    cell["ru"] = ru_ps

                def add():
                    dstq = (qT_sb, kT_sb)[wn_idx]
                    nc.vector.tensor_add(dstq[:, m, cell["sl"]],
                                         cell["t1"], cell["ru"])
                return main, swap_mm, add

            def emit_1b(t16, evict_act=False):
                n, s_ = t16 // 4, t16 % 4
                x_ch = x_tiles[n]
                ps = psqk.tile([128, BLK], fp32, name=f"psv{t16}", tag="qk")
                for k in range(16):
                    nc.tensor.matmul(ps,
                                     lhsT=x_ch[:, k, s_ * 128:(s_ + 1) * 128],
                                     rhs=wv_sb[:, k, :],
                                     start=(k == 0), stop=(k == 15))
                dstv = v_sb[:, t16, :].rearrange("p (h c) -> p h c", c=65)[:, :, 0:64]
                psv = ps.rearrange("p (h c) -> p h c", c=64)
                if evict_act:
                    nc.scalar.copy(dstv, psv)
                else:
                    nc.vector.tensor_copy(dstv, psv)

            def chunk_thunks(n, prologue=False):
                if prologue:
                    # PE-swap variant: emission order per step i is
                    # [main(i), add(i-2), swap_mm(i-1)] so the psav slot
                    # writer (swap_mm) is always emitted after the previous
                    # generation's reader (add) -- bufs=1 slot rotation.
                    mains, swaps, adds = [], [], []
                    for wn_idx in range(2):
                        for m in range(NPR):
                            a, b, c = make_1a_pe(wn_idx, m, n)
                            mains.append(a)
                            swaps.append(b)
                            adds.append(c)
                    th = []
                    for i, mth in enumerate(mains):
                        th.append(mth)
                        if i >= 2:
                            th.append(adds[i - 2])
                        if i >= 1:
                            th.append(swaps[i - 1])
                    tail_ops = [swaps[7], adds[6], adds[7]]
                    for t16 in range(4 * n, 4 * n + 4):
                        # ACT is idle in the prologue (no exps yet) --
                        # evict V there.
                        th.append(lambda t=t16: emit_1b(t, True))
                        if tail_ops:
                            th.append(tail_ops.pop(0))
                    th.extend(tail_ops)
                    return th
                mains, adds = [], []
                for wn_idx in range(2):
                    for m in range(NPR):
                        a, b = make_1a(wn_idx, m, n)
                        mains.append(a)
                        adds.append(b)
                for t16 in range(4 * n, 4 * n + 4):
                    mains.append(lambda t=t16: emit_1b(t, False))
                # interleave with the GPS adds lagging their thunk by 2
                th, ai = [], 0
                for i, mth in enumerate(mains):
                    th.append(mth)
                    if ai < len(adds) and ai <= i - 2:
                        th.append(adds[ai])
                        ai += 1
                th.extend(adds[ai:])
                return th

            def emit_outproj_half(t16, no, evict_act=False, pool_sel=0,
                                  store_eng=None):
                # one [128,512] output column block (1 psum bank) on the
                # same "qk" tag the projection thunks use -- their
                # lifetimes are complementary (projections fill columns
                # 0-2, out-projection fills column 3 + tail).  The final
                # 16 halves rotate across qk/av/s tags (all idle then) so
                # psum slot reuse never waits on an eviction.
                pool_, tag_ = ((psqk, "qk"), (psav, "av"), (pss, "s"))[pool_sel]
                o_ps = pool_.tile([128, 512], fp32,
                                  name=f"o{t16}_{no}", tag=tag_)
                for pr_ in range(NPR):
                    nc.tensor.matmul(
                        o_ps,
                        lhsT=attn_sb[:, pr_, t16 * 128:(t16 + 1) * 128],
                        rhs=wo_sb[:, pr_, no * 512:(no + 1) * 512],
                        start=(pr_ == 0), stop=(pr_ == NPR - 1))
                o_sb = ostage.tile([128, 512], bf16,
                                   name=f"os{t16}_{no}", tag="os")
                if evict_act:
                    nc.scalar.copy(o_sb, o_ps)
                else:
                    nc.vector.tensor_copy(o_sb, o_ps)
                (store_eng or nc.sync).dma_start(
                    out=out[t16 * 128:(t16 + 1) * 128,
                            no * 512:(no + 1) * 512],
                    in_=o_sb)

            fill_q = deque()
            defer_q = deque()

            def drain_one():
                if fill_q:
                    fill_q.popleft()()

            def flush():
                while fill_q:
                    fill_q.popleft()()

            # ---------------- attention ----------------
            def attend(pr, j):
                n_i = 4 * j + 4
                av = psav.tile([128, 1024], fp32, name=f"av_{pr}_{j}", tag="av")
                s_t, p_t, nw_t = {}, {}, {}

                def emit_S(i):
                    off = max(0, 128 * (i - 4 * j))
                    nw = 512 - off
                    nw_t[i] = (off, nw)
                    s_ps = pss.tile([128, 1024], fp32,
                                    name=f"s_{pr}_{j}_{i}", tag="s")
                    for half in range(2):
                        r0 = 64 * half
                        nc.tensor.matmul(
                            s_ps[:, half * 512:half * 512 + nw],
                            lhsT=kT_sb[r0:r0 + 64, pr, i * 128:(i + 1) * 128],
                            rhs=qT_sb[r0:r0 + 64, pr, j * 512 + off:(j + 1) * 512],
                            start=True, stop=True,
                            tile_position=(r0, 0))
                    s_t[i] = s_ps

                def emit_exp(i):
                    off, nw = nw_t[i]
                    s_ps = s_t[i]
                    s_v = s_ps.rearrange("q (h t) -> q h t", h=2)[:, :, 0:nw]
                    p = ppool.tile([128, 1024], SDT,
                                   name=f"p_{pr}_{j}_{i}", tag="p")
                    p_v = p.rearrange("q (h t) -> q h t", h=2)[:, :, 0:nw]
                    nc.scalar.activation(p_v, s_v,
                                         mybir.ActivationFunctionType.Exp,
                                         scale=0.125)
                    if i >= 4 * j:
                        nc.gpsimd.affine_select(
                            out=p_v, in_=p_v,
                            compare_op=mybir.AluOpType.is_ge,
                            fill=0.0,
                            base=512 * j + off - 128 * i,
                            pattern=[[0, 2], [1, nw]],
                            channel_multiplier=-1)
                    p_t[i] = p

                def emit_AV(i, first, last):
                    off, nw = nw_t[i]
                    p = p_t[i]
                    for half in range(2):
                        h = 2 * pr + half
                        nc.tensor.matmul(
                            av[0:65, half * 512 + off:half * 512 + 512],
                            lhsT=v_sb[:, i, 65 * h:65 * h + 65],
                            rhs=p[:, half * 512:half * 512 + nw],
                            start=first, stop=last)

                order = list(range(n_i))
                first_i, last_i = order[0], order[-1]
                emit_S(order[0])
                if n_i > 1:
                    emit_S(order[1])
                emit_exp(order[0])
                for ii, i in enumerate(order):
                    if ii + 2 < n_i:
                        emit_S(order[ii + 2])
                    if ii + 1 < n_i:
                        emit_exp(order[ii + 1])
                    emit_AV(i, i == first_i, i == last_i)
                    # column 3 has 48 filler thunks for 64 iterations --
                    # pace them 3-of-4 so the filler lasts the whole column.
                    # One extra skip per attend (ii == n_i-2) leaves 4
                    # halves (~3.6us of PE work) undrained until the
                    # post-attend flush, which lands them exactly in the
                    # final-normalize 1/Z window where PE otherwise idles.
                    if not (j == NCH - 1 and (ii % 4 == 3 or ii == n_i - 2)):
                        drain_one()

                # normalize (see build_program attend for rationale)
                jsl = slice(j * 512, (j + 1) * 512)
                if pr == NPR - 1 and j == NCH - 1:
                    # The very last normalize gates the final out-projection:
                    # skip the DMA bounces, 1/Z = exp(-ln Z) on ACT.
                    # Latency trims: (1) a dummy Ln READING THE LAST P TILE
                    # (the data dep pins it after the final exp -- an
                    # input-free dummy gets hoisted by the scheduler before
                    # the remaining exps and thrashes the table) preloads
                    # the Ln table during the last AVs; (2) Ln reads the
                    # denominator row straight from the av PSUM bank, so it
                    # starts at the last AV, not after the raw evict (which
                    # runs on DVE in parallel); (3) per-half Ln/Exp with lg
                    # staged in SBUF so each broadcast matmul + attn mul
                    # fires half an Exp earlier and no PSUM slot is held.
                    dmy = bcpool.tile([1, 64], bf16, name="lnwarm", tag="bc1")
                    nc.scalar.activation(dmy, p_t[n_i - 1][0:1, 0:64],
                                         mybir.ActivationFunctionType.Ln)
                    raw = npool.tile([65, 1024], fp32,
                                     name=f"raw_{pr}_{j}", tag="raw")
                    nc.vector.tensor_copy(raw[0:64, :], av[0:64, :])
                    lgs = []
                    for half in range(2):
                        hs = slice(half * 512, half * 512 + 512)
                        lg = npool.tile([1, 512], fp32,
                                        name=f"lg{half}_last", tag="lg")
                        nc.scalar.activation(lg, av[64:65, hs],
                                             mybir.ActivationFunctionType.Ln)
                        lgs.append(lg)
                    rec = bcpool.tile([1, 1024], bf16,
                                      name="rec_last", tag="bc0")
                    for half in range(2):
                        hs = slice(half * 512, half * 512 + 512)
                        nc.scalar.activation(rec[:, hs], lgs[half],
                                             mybir.ActivationFunctionType.Exp,
                                             scale=-1.0)
                        # bc_ps on the "av" slot (free once the Ln PSUM
                        # reads retire) so both "qk" slots stay available
                        # for early tail-chain opens
                        bc_ps = psav.tile([64, 512], fp32,
                                          name=f"bcp{half}", tag="av")
                        nc.tensor.matmul(bc_ps, lhsT=ones1b_sb, rhs=rec[:, hs],
                                         start=True, stop=True)
                        nc.vector.tensor_mul(
                            attn_sb[64 * half:64 * half + 64, pr, jsl],
                            raw[0:64, hs], bc_ps)
                    return
                raw = npool.tile([65, 1024], fp32, name=f"raw_{pr}_{j}", tag="raw")
                if pr == NPR - 1:
                    # boundary: the NEXT column's first AV waits on this
                    # eviction (psav bufs=1).  The DVE FIFO is congested
                    # here (filler rope muls + normalize muls, ~0.7us each),
                    # so evict on ACT, whose queue has drained its exps by
                    # the last AV -- saves a ~3-6us PE stall per boundary.
                    nc.scalar.copy(raw, av[0:65, :])
                else:
                    nc.vector.tensor_copy(raw, av[0:65, :])
                den_d = dpool.tile([1, 1024], fp32, name=f"dd_{pr}_{j}", tag="dd")
                nc.sync.dma_start(out=den_d, in_=raw[64:65, :])
                den_t = npool.tile([128, 8], fp32, name=f"dt_{pr}_{j}", tag="dt")
                nc.sync.dma_start(
                    out=den_t, in_=den_d.rearrange("o (p c) -> (o p) c", c=8))
                rec_t = npool.tile([128, 8], fp32, name=f"rt_{pr}_{j}", tag="rt")
                nc.vector.reciprocal(rec_t, den_t)
                rec_d = dpool.tile([128, 8], fp32, name=f"rd_{pr}_{j}", tag="rd")
                nc.sync.dma_start(out=rec_d, in_=rec_t)
                bcs = []
                for half in range(2):
                    bc = bcpool.tile([64, 512], fp32,
                                     name=f"bc{half}_{pr}_{j}", tag=f"bc{half}")
                    bc_src = bass.AP(
                        tensor=rec_d.tensor, offset=rec_d.offset + half * 512,
                        ap=[[0, 64], [1, 512]])
                    nc.sync.dma_start(out=bc, in_=bc_src)
                    bcs.append(bc)

                # (Deferring pr3's muls into the next column's filler
                # stream was tried to dodge the DVE head-of-line wait on
                # the bc DMA at boundaries -- measured neutral-to-worse,
                # so they stay inline.  Deferring pr0-2 would deadlock the
                # npool raw rotation outright: with bufs=2 the pr+2 evict,
                # earlier in the DVE program, waits a mul emitted behind
                # it on the same FIFO.)
                for half in range(2):
                    nc.vector.tensor_mul(
                        attn_sb[64 * half:64 * half + 64, pr, jsl],
                        raw[0:64, half * 512:half * 512 + 512], bcs[half])

            # ---------------- main schedule ----------------
            # Columns 0-2 are PE-rich (next chunk's projections fill them);
            # column 3 has the most exp (ACT-bound) and no projections left,
            # so ALL of columns 0-2's out-projection is deferred there as
            # its PE filler.  Column 3's outproj lands after the loop.
            # x is prefetched two columns ahead so projection filler thunks
            # never wait on their input transfer.
            for th in chunk_thunks(0, prologue=True):
                th()
            for n in range(NCH):
                if n + 2 < NCH:
                    load_x(n + 2)
                if n == NCH - 1:
                    for t16 in range(0, 12):
                        for no in range(4):
                            fill_q.append(
                                lambda t=t16, o=no: emit_outproj_half(t, o))
                if n + 1 < NCH:
                    fill_q.extend(chunk_thunks(n + 1))
                for pr in range(NPR):
                    attend(pr, n)
                flush()
                # deferred normalize muls drain as next-column fillers
                fill_q.extend(defer_q)
                defer_q.clear()
            # column 3's deferred muls (prs 0-2) must land before the tail
            # closes read attn_sb
            flush()
            # Final 16 outproj halves, software-pipelined: each chain's
            # pr0-2 matmuls are emitted 3 chains ahead of its pr3 matmul +
            # evict, so the pr3-gating dependency (the last normalize's
            # 1/Z chain) hides behind ~2us of pr0-2 matmul work instead of
            # stalling the strict-FIFO PE queue.  Slot tags are ordered so
            # the first opens use banks that are free at the last AV ("av",
            # "s"), not "qk" (still held by the final normalize's bc_ps).
            # slot order: the first four opens use the pss "s" slots (free
            # once the last exps consumed them) and the psqk "qk" slots
            # (freed by the last fillers' prompt evicts) -- all unblocked
            # at the last AV, giving ~2.6us of pr0-2 matmul cover over the
            # final 1/Z chain.  "av" comes last: it is held by the Ln PSUM
            # reads and then the bc_ps tiles (placing an open before bc_ps
            # on the same tag would deadlock: the open's close needs the
            # attn mul, which needs bc_ps, which would wait on the open).
            tail = [(t, o) for t in range(12, 16) for o in range(4)]
            tpools = ((pss, "s"), (pss, "s"), (psqk, "qk"),
                      (psqk, "qk"), (psav, "av"))
            opened = {}

            def open_chain(idx):
                t16, no = tail[idx]
                pool_, tag_ = tpools[idx % 5]
                o_ps = pool_.tile([128, 512], fp32,
                                  name=f"o{t16}_{no}", tag=tag_)
                for pr_ in range(NPR - 1):
                    nc.tensor.matmul(
                        o_ps,
                        lhsT=attn_sb[:, pr_, t16 * 128:(t16 + 1) * 128],
                        rhs=wo_sb[:, pr_, no * 512:(no + 1) * 512],
                        start=(pr_ == 0), stop=False)
                opened[idx] = o_ps

            def close_chain(idx):
                t16, no = tail[idx]
                o_ps = opened.pop(idx)
                nc.tensor.matmul(
                    o_ps,
                    lhsT=attn_sb[:, NPR - 1, t16 * 128:(t16 + 1) * 128],
                    rhs=wo_sb[:, NPR - 1, no * 512:(no + 1) * 512],
                    start=False, stop=True)
                o_sb = ostage.tile([128, 512], bf16,
                                   name=f"os{t16}_{no}", tag="os")
                if no % 2 == 0:
                    nc.scalar.copy(o_sb, o_ps)
                else:
                    nc.vector.tensor_copy(o_sb, o_ps)
                (nc.scalar, nc.sync)[idx % 2].dma_start(
                    out=out[t16 * 128:(t16 + 1) * 128,
                            no * 512:(no + 1) * 512],
                    in_=o_sb)

            for idx in range(4):
                open_chain(idx)
            for idx in range(len(tail)):
                close_chain(idx)
                if idx + 4 < len(tail):
                    open_chain(idx + 4)
    nc.compile()
    return nc


INTERLEAVE = True


def _get_program(variant, dtype):
    key = (variant, dtype, INTERLEAVE)
    if key not in _PROGRAMS:
        if variant == "causal" and dtype == "bf16" and INTERLEAVE:
            _PROGRAMS[key] = build_program_il(dtype)
        else:
            _PROGRAMS[key] = build_program(variant, dtype)
    return _PROGRAMS[key]


def _np_dt(dtype):
    return ml_dtypes.bfloat16 if dtype == "bf16" else np.float32


def make_in_maps(hidden_states, attention_mask, position_ids, Wq, Wk, Wv, Wo,
                 variant, dtype):
    npdt = _np_dt(dtype)
    cos, sin = _rope_cache()
    in_maps = []
    for c in range(N_CORES):
        b = c // (N_CORES // 2)
        hb = c % (N_CORES // 2)
        rs = slice(hb * BLK, (hb + 1) * BLK)
        pos = np.asarray(position_ids[b]).astype(np.int64)
        cos_b = cos[pos].T.astype(np.float32)     # [64, S]
        sin_b = sin[pos].T.astype(np.float32)
        # sign-folded sin for rotate-half...
        sin_s = np.concatenate([-sin_b[:32], sin_b[32:]], axis=0)
        # ...then partition-swapped (32-row blocks) so the kernel can apply
        # the rotate-half swap AFTER the multiply via a plain DMA copy:
        # sin_sh[e] = sin_s[swap(e)]
        sin_sh = np.concatenate([sin_s[32:64], sin_s[0:32]], axis=0)
        m = {
            "xT": np.ascontiguousarray(np.asarray(hidden_states)[b].T).astype(npdt),
            "wqT": np.ascontiguousarray(np.asarray(Wq)[rs].T).astype(npdt),
            "wkT": np.ascontiguousarray(np.asarray(Wk)[rs].T).astype(npdt),
            "wvT": np.ascontiguousarray(np.asarray(Wv)[rs].T).astype(npdt),
            "woT": np.ascontiguousarray(np.asarray(Wo)[:, rs].T).astype(npdt),
        }
        cos_full = np.ascontiguousarray(np.concatenate([cos_b, cos_b], axis=0))
        sin_full = np.ascontiguousarray(np.concatenate([sin_sh, sin_sh], axis=0))
        if variant == "causal" and dtype == "bf16" and INTERLEAVE:
            m["cosb"] = cos_full.astype(ml_dtypes.bfloat16)
            m["sinb"] = sin_full.astype(ml_dtypes.bfloat16)
            # rotate-half 32-row block-swap permutation: pm[k, j] = 1 iff
            # k == swap(j); (pm.T @ u)[j] = u[swap(j)]
            pm = np.zeros((128, 128), dtype=np.float32)
            swap_idx = np.concatenate([np.arange(32, 64), np.arange(0, 32),
                                       np.arange(96, 128), np.arange(64, 96)])
            pm[swap_idx, np.arange(128)] = 1.0
            m["pm"] = pm.astype(ml_dtypes.bfloat16)
        else:
            m["cos2"] = cos_full
            m["sin2s"] = sin_full
        if variant == "general":
            m["maskT"] = np.ascontiguousarray(
                np.asarray(attention_mask)[b, 0].T).astype(ml_dtypes.bfloat16)
        in_maps.append(m)
    return in_maps


def detect_causal(attention_mask):
    am = np.asarray(attention_mask)
    if am.shape != (2, 1, S, S):
        return False
    neg = np.float32(np.finfo(np.float32).min)
    canonical = np.where(np.tril(np.ones((S, S), dtype=bool)), np.float32(0.0), neg)
    return bool(np.array_equal(am[0, 0], canonical) and
                np.array_equal(am[1, 0], canonical))


DTYPE = "bf16"


def kernel(hidden_states, attention_mask, position_ids, Wq, Wk, Wv, Wo):
    hidden_states = np.asarray(hidden_states, dtype=np.float32)
    attention_mask = np.asarray(attention_mask, dtype=np.float32)
    Wq, Wk, Wv, Wo = (np.asarray(w, dtype=np.float32) for w in (Wq, Wk, Wv, Wo))

    variant = "causal" if detect_causal(attention_mask) else "general"
    nc = _get_program(variant, DTYPE)
    in_maps = make_in_maps(hidden_states, attention_mask, position_ids,
                           Wq, Wk, Wv, Wo, variant, DTYPE)

    from concourse import bass2jax
    results = bass2jax.run_bass_via_pjrt(nc, in_maps, n_cores=N_CORES)

    out = np.zeros((2, S, HID), dtype=np.float64)
    for c in range(N_CORES):
        b = c // (N_CORES // 2)
        out[b] += results[c]["out"].astype(np.float64)
    return out.astype(np.float32)

